# revision 9
# baseline (speedup 1.0000x reference)
"""Trainium2 Bass kernel for NCM/kNN retrieval (nn_NCM_30468497998426).

Structure per core (queries 8-way sharded, support replicated):
  - host casts support to bf16; DMA'd once as [125, RPP*512] tiles (RPP
    support rows per partition -> RPP*1KB DMA packets, amortizing the
    ~100GB/s packet-rate-bound DMA).
  - stream phase (no mu dependency): PE transposes raw bf16 support into
    AT[k] [128, 5000] (cs in natural order via stride-RPP copybacks);
    gpsimd squares + vector reduces -> row norms; gpsimd adds -> mean.
  - algebraic centering/normalization (mu available after the stream):
      sims[cs,q] = inv[cs]*(A[cs]·qn[q]) - inv[cs]*(mu·qn[q])
      inv[cs] = rsqrt(||A[cs]||^2 - 2 A[cs]·mu + ||mu||^2)
    AT is scaled in place by inv (row broadcast); the rank-1 correction is
    a K=1 matmul into the same PSUM accumulation group.
  - mains in bf16 (PE at 1 cycle/col), shots-max on DVE, top-8 per query.
  - host re-ranks low-margin queries (top1-top2 < MARGIN_T, ~4%) exactly
    in float64 among their top-8 candidate classes.
"""

import numpy as np

import concourse.bacc as bacc
import concourse.mybir as mybir
import concourse.tile as tile
from concourse.alu_op_type import AluOpType
from concourse.bass_utils import run_bass_kernel_spmd

F32 = mybir.dt.float32
F32R = mybir.dt.float32r
BF16 = mybir.dt.bfloat16
I32 = mybir.dt.int32
U32 = mybir.dt.uint32
AF = mybir.ActivationFunctionType

C, S, D = 1000, 5, 512
CS = C * S
Q = 5000
NCORES = 8
QS = Q // NCORES        # 625
P = 125
KC = D // 128           # 4
QT = QS // P            # 5
CSCH = 500
NJ = CS // CSCH         # 10
GPC = CSCH // S         # 100

RPP = 4                 # support rows per partition in the DMA tiles
W = RPP * D             # tile width
NT = CS // (P * RPP)    # support tiles (20 at RPP=2)
TPJ = CSCH // (P * RPP)  # tiles per cs chunk (2)

MARGIN_T = 1e-3
DEEP_T = 1e-3


def build():
    nc = bacc.Bacc(None, target_bir_lowering=False)

    sup = nc.declare_dram_parameter("support", [CS // RPP, W], BF16,
                                    isOutput=False)
    qry = nc.declare_dram_parameter("queries", [QS, D], F32, isOutput=False)
    ident = nc.declare_dram_parameter("ident", [128, 128], F32, isOutput=False)
    ones_col = nc.declare_dram_parameter("ones_col", [128, 1], F32, isOutput=False)
    ones_row = nc.declare_dram_parameter("ones_row", [1, 128], F32, isOutput=False)
    out = nc.declare_dram_parameter("out", [QS, 1], I32, isOutput=True)
    out_mx8 = nc.declare_dram_parameter("out_mx8", [QS, 8], F32, isOutput=True)
    out_ix8 = nc.declare_dram_parameter("out_ix8", [QS, 8], I32, isOutput=True)

    flip = [0]

    def r(ap):
        return ap.bitcast(F32R)

    def copyback(dst, src):
        # rotate vector/scalar (gpsimd cannot read PSUM)
        if flip[0] % 2 == 0:
            nc.vector.tensor_copy(dst, src)
        else:
            nc.scalar.copy(dst, src)
        flip[0] += 1

    with tile.TileContext(nc) as tc:
        with (
            tc.tile_pool(name="const", bufs=1) as pconst,
            tc.tile_pool(name="stat", bufs=1) as pstat,
            tc.tile_pool(name="at", bufs=1) as pat,
            tc.tile_pool(name="qt", bufs=1) as pqt,
            tc.tile_pool(name="anat", bufs=6) as pa,
            tc.tile_pool(name="qnat", bufs=QT) as pq,
            tc.tile_pool(name="macc", bufs=1) as pmacc,
            tc.tile_pool(name="scratch", bufs=2) as pscr,
            tc.tile_pool(name="rows", bufs=2) as prows,
            tc.tile_pool(name="best", bufs=1) as pbest,
            tc.tile_pool(name="res", bufs=2) as pres,
            tc.tile_pool(name="invb", bufs=2) as pinvb,
            tc.tile_pool(name="trpsum", bufs=1, space="PSUM") as ptr,
            tc.tile_pool(name="r1psum", bufs=1, space="PSUM") as pr1,
            tc.tile_pool(name="b128psum", bufs=1, space="PSUM") as pb1,
            tc.tile_pool(name="mmpsum", bufs=1, space="PSUM") as pmm,
        ):
            id_sb = pconst.tile([128, 128], F32, tag="ident")
            nc.sync.dma_start(id_sb[:], ident[:])
            idb = pconst.tile([128, 128], BF16, tag="identb")
            nc.vector.tensor_copy(idb[:], id_sb[:])
            onec_sb = pconst.tile([128, 1], F32, tag="onec")
            nc.sync.dma_start(onec_sb[:], ones_col[:])
            oner_sb = pconst.tile([1, 128], F32, tag="oner")
            nc.sync.dma_start(oner_sb[:], ones_row[:])
            onerb = pconst.tile([1, 128], BF16, tag="onerb")
            nc.vector.tensor_copy(onerb[:], oner_sb[:])
            oner_r = pconst.tile([1, 128], F32, tag="oner_r")
            nc.vector.tensor_copy(r(oner_r[:]), oner_sb[:])

            at_tiles = [pat.tile([128, CS], BF16, name=f"at{k}", tag=f"at{k}")
                        for k in range(KC)]
            qt_tiles = [pqt.tile([128, QS], BF16, name=f"qt{k}", tag=f"qt{k}")
                        for k in range(KC)]
            qk_row = pstat.tile([1, QS], BF16, tag="qk_row")
            neginv_all = pstat.tile([1, CS], BF16, tag="neginv")
            n2all = pstat.tile([P, CS // P], F32, tag="n2all")

            onecb = pconst.tile([128, 1], BF16, tag="onecb")
            nc.vector.tensor_copy(onecb[:], onec_sb[:])

            slot_ctr = [0, 0]

            def tr_slot():
                slot_ctr[0] += 1
                t = ptr.tile([128, P], BF16, tag="tp", bufs=3,
                             name=f"tp{slot_ctr[0]}")
                return t[:]

            def trf_slot():
                slot_ctr[1] += 1
                t = ptr.tile([128, P], F32, tag="tpf", bufs=1,
                             name=f"tpf{slot_ctr[1]}")
                return t[:]

            # ---- stream support once (bf16): transpose + norms + mean.
            # Host pre-permutes each 500-row block (DRAM row 4p+h holds true
            # row 125h+p) so the transposed copybacks are CONTIGUOUS while cs
            # stays in natural order for the shots-max.
            acc = pmacc.tile([P, W], F32, tag="acc")
            acc2 = pmacc.tile([P, W], F32, tag="acc2")
            with nc.named_scope("stream"):
                def consume(t, src):
                    for h in range(RPP):
                        # row norms ||A||^2 for this half -> n2all column
                        hs = src[:, h * D:(h + 1) * D]
                        n2dst = n2all[:, t * RPP + h:t * RPP + h + 1]
                        sq = pscr.tile([P, D], F32, tag="sq", bufs=3)
                        nc.scalar.activation(sq[:], hs, AF.Square,
                                             accum_out=n2dst)
                        for kk in range(KC):
                            tp = tr_slot()
                            nc.tensor.transpose(
                                tp,
                                src[:, h * D + kk * 128: h * D + (kk + 1) * 128],
                                idb[0:P, 0:P])
                            base = t * P * RPP + h * P
                            copyback(at_tiles[kk][:, base:base + P], tp)

                for t in range(NT):
                    lt = pa.tile([P, W], BF16, tag="a")
                    nc.sync.dma_start(lt[:], sup[t * P:(t + 1) * P, :])
                    if t == 0:
                        nc.gpsimd.tensor_copy(acc[:], lt[:])
                    elif t == 1:
                        nc.gpsimd.tensor_copy(acc2[:], lt[:])
                    else:
                        a_dst = acc if t % 2 == 0 else acc2
                        nc.gpsimd.tensor_add(a_dst[:], a_dst[:], lt[:])
                    consume(t, lt)

            # ---- q loads (after support: support DMA fills the stream)
            q_tiles = []
            with nc.named_scope("load"):
                for i in range(QT):
                    qt_ = pq.tile([P, D], F32, tag="q", bufs=QT)
                    nc.sync.dma_start(qt_[:], qry[i * P:(i + 1) * P, :])
                    q_tiles.append(qt_)

            # ---- mean
            with nc.named_scope("mean"):
                nc.gpsimd.tensor_add(acc[:], acc[:], acc2[:])
                mu_ps = pr1.tile([1, D], F32, tag="r1", bufs=1)
                for h in range(RPP):
                    nc.tensor.matmul(mu_ps[:], onec_sb[0:P, :],
                                     acc[:, h * D:(h + 1) * D],
                                     start=(h == 0), stop=(h == RPP - 1))
                mu_sb = pstat.tile([1, D], F32, tag="mu_sb")
                nc.vector.tensor_scalar_mul(mu_sb[:], mu_ps[:], 1.0 / CS)
                mub_ps = pb1.tile([128, D], F32, tag="b128", bufs=1)
                nc.tensor.matmul(mub_ps[:], oner_sb[:], mu_sb[:],
                                 start=True, stop=True)
                mu_b = pstat.tile([128, D], F32, tag="mu_b")
                nc.vector.tensor_copy(mu_b[:], mub_ps[:])
                # mu column chunks (bf16) for the A·mu matvecs
                mucol = pstat.tile([128, KC], BF16, tag="mucol")
                for k in range(KC):
                    tpm = trf_slot()
                    nc.tensor.transpose(tpm[:, 0:1],
                                        mu_sb[0:1, k * 128:(k + 1) * 128],
                                        id_sb[0:1, 0:1])
                    nc.vector.tensor_copy(mucol[:, k:k + 1], tpm[:, 0:1])
                sqm = pscr.tile([1, D], F32, tag="sqm")
                mu2 = pstat.tile([1, 1], F32, tag="mu2")
                nc.scalar.activation(sqm[:], mu_sb[:], AF.Square,
                                     accum_out=mu2[:])

            # ---- query side: center, normalize(+cast), transpose, qK
            with nc.named_scope("qside"):
                for i in range(QT):
                    qt_ = q_tiles[i]
                    nc.vector.tensor_sub(qt_[:], qt_[:], mu_b[0:P, :])
                    sqq = pscr.tile([P, D], F32, tag="sqq")
                    n2q = prows.tile([P, 1], F32, tag="n2q", bufs=2)
                    nc.scalar.activation(sqq[:], qt_[:], AF.Square,
                                         accum_out=n2q[:])
                    nrmq = prows.tile([P, 1], F32, tag="nrmq", bufs=2)
                    nc.scalar.activation(nrmq[:], n2q[:], AF.Sqrt)
                    invq = prows.tile([P, 1], F32, tag="invq", bufs=2)
                    nc.vector.reciprocal(invq[:], nrmq[:])
                    qb = pscr.tile([P, D], BF16, tag="qb")
                    nc.scalar.activation(qb[:], qt_[:], AF.Copy,
                                         scale=invq[:])
                    for k in range(KC):
                        tp = tr_slot()
                        nc.tensor.transpose(tp,
                                            qb[:, k * 128:(k + 1) * 128],
                                            idb[0:P, 0:P])
                        copyback(qt_tiles[k][:, i * P:(i + 1) * P], tp)
                    # qK = mu·qn (needs the normalized q: scale afterwards)
                    trash = pscr.tile([P, D], F32, tag="trash")
                    qkc = prows.tile([P, 1], F32, tag="qkc", bufs=2)
                    nc.vector.scalar_tensor_tensor(
                        out=trash[:], in0=qt_[:], scalar=1.0,
                        in1=mu_b[0:P, :],
                        op0=AluOpType.bypass, op1=AluOpType.mult,
                        accum_out=qkc[:])
                    qkn = prows.tile([P, 1], F32, tag="qkn", bufs=2)
                    nc.vector.tensor_mul(qkn[:], qkc[:], invq[:])
                    tpq = trf_slot()
                    nc.tensor.transpose(tpq[0:1, 0:P], qkn[:], id_sb[0:P, 0:P])
                    nc.vector.tensor_copy(qk_row[0:1, i * P:(i + 1) * P],
                                          tpq[0:1, 0:P])

            # ---- norms for all chunks: inv -> scale AT in place
            for j in range(NJ):
                jsl = slice(j * CSCH, (j + 1) * CSCH)
                with nc.named_scope(f"norm{j}"):
                    # n2 row layout [1,500] in natural cs order (one
                    # single-column transpose per n2all column: engines
                    # cannot read PSUM at a nonzero base partition)
                    nrow = CS // P // NJ  # n2all cols per chunk (4)
                    n2cr = prows.tile([1, CSCH], F32, tag="n2cr", bufs=2)
                    for c in range(nrow):
                        tpn = trf_slot()
                        nc.tensor.transpose(
                            tpn[0:1, :], n2all[:, j * nrow + c:j * nrow + c + 1],
                            id_sb[0:P, 0:P])
                        h = c  # one tile per chunk at RPP=4
                        copyback(n2cr[0:1, h * P:(h + 1) * P], tpn[0:1, 0:P])
                    amu = pr1.tile([1, D], F32, tag="r1", bufs=1)
                    for k in range(KC):
                        nc.tensor.matmul(amu[:, 0:CSCH], mucol[:, k:k + 1],
                                         at_tiles[k][:, jsl],
                                         start=(k == 0), stop=(k == KC - 1))
                    n2c = prows.tile([1, CSCH], F32, tag="n2c", bufs=2)
                    nc.vector.scalar_tensor_tensor(
                        out=n2c[:], in0=amu[:, 0:CSCH], scalar=-2.0,
                        in1=n2cr[:],
                        op0=AluOpType.mult, op1=AluOpType.add)
                    nc.vector.tensor_scalar_add(n2c[:], n2c[:], mu2[0:1, 0:1])
                    nc.scalar.activation(n2c[:], n2c[:], AF.Sqrt)
                    invf = prows.tile([1, CSCH], F32, tag="invf", bufs=2)
                    nc.vector.reciprocal(invf[:], n2c[:])
                    invt = prows.tile([1, CSCH], F32, tag="invt", bufs=2)
                    with nc.allow_low_precision(
                            reason="13-bit fp32r inv is intentional"):
                        nc.vector.tensor_scalar_mul(r(invt[:]), invf[:], 1.0)
                        nc.vector.tensor_scalar_mul(neginv_all[0:1, jsl],
                                                    invf[:], -1.0)
                    # broadcast inv across partitions at 13-bit (coherent
                    # per-row scale error must stay well under bf16)
                    bps = pb1.tile([128, D], F32, tag="b128", bufs=1)
                    nc.tensor.matmul(bps[:, 0:CSCH], r(oner_r[:]),
                                     r(invt[:]), start=True, stop=True)
                    invb = pinvb.tile([128, CSCH], F32, tag="invb")
                    nc.vector.tensor_copy(invb[:, 0:250], bps[:, 0:250])
                    nc.scalar.copy(invb[:, 250:CSCH], bps[:, 250:CSCH])
                    for k in range(KC):
                        eng = nc.vector if k % 2 == 0 else nc.gpsimd
                        eng.tensor_mul(at_tiles[k][:, jsl],
                                       at_tiles[k][:, jsl], invb[:])

            # ---- mains
            best_tiles = [pbest.tile([P, C], F32, name=f"best{i}",
                                     tag=f"best{i}") for i in range(QT)]
            for j in range(NJ):
                jsl = slice(j * CSCH, (j + 1) * CSCH)
                with nc.named_scope(f"mm{j}"):
                    for i in range(QT):
                        isl = slice(i * P, (i + 1) * P)
                        ps = pmm.tile([P, CSCH], F32, tag="sims", bufs=2)
                        for k in range(KC):
                            nc.tensor.matmul(
                                ps[:], qt_tiles[k][:, isl],
                                at_tiles[k][:, jsl],
                                start=(k == 0), stop=False)
                        nc.tensor.matmul(
                            ps[:], qk_row[0:1, isl], neginv_all[0:1, jsl],
                            start=False, stop=True)
                        dst = best_tiles[i][:, j * GPC:(j + 1) * GPC]
                        nc.vector.tensor_reduce(
                            out=dst,
                            in_=ps[:].rearrange("p (c s) -> p c s", s=S),
                            axis=mybir.AxisListType.X, op=AluOpType.max,
                        )

            # ---- top-8 over classes
            with nc.named_scope("argmax"):
                for i in range(QT):
                    mx8 = pres.tile([P, 8], F32, tag="mx8")
                    ix8 = pres.tile([P, 8], U32, tag="ix8")
                    nc.vector.max_with_indices(mx8[:], ix8[:], best_tiles[i][:])
                    ii = pres.tile([P, 1], I32, tag="ii")
                    nc.vector.tensor_copy(ii[:], ix8[:, 0:1])
                    nc.sync.dma_start(out[i * P:(i + 1) * P, :], ii[:])
                    ix8s = pres.tile([P, 8], I32, tag="ix8s")
                    nc.vector.tensor_copy(ix8s[:], ix8[:])
                    nc.sync.dma_start(out_mx8[i * P:(i + 1) * P, :], mx8[:])
                    nc.sync.dma_start(out_ix8[i * P:(i + 1) * P, :], ix8s[:])

    nc.finalize()
    return nc


def _host_inputs(support_features, query_features):
    import ml_dtypes
    sup32 = np.ascontiguousarray(
        np.asarray(support_features, dtype=np.float32).reshape(CS, D))
    # permute each 500-row block: DRAM row 4p+h holds true row 125h+p, so
    # the device's per-half transposes land contiguously in natural cs order
    blk = CSCH
    j = np.arange(blk)
    perm = (j % RPP) * P + j // RPP
    sup = sup32.reshape(CS // blk, blk, D)[:, perm, :].reshape(CS, D) \
        .astype(ml_dtypes.bfloat16).reshape(CS // RPP, W)
    qf = np.ascontiguousarray(np.asarray(query_features, dtype=np.float32))
    ident = np.eye(128, dtype=np.float32)
    ones_col = np.ones((128, 1), dtype=np.float32)
    ones_row = np.ones((1, 128), dtype=np.float32)
    in_maps = []
    for c in range(NCORES):
        in_maps.append({
            "support": sup,
            "queries": np.ascontiguousarray(qf[c * QS:(c + 1) * QS]),
            "ident": ident,
            "ones_col": ones_col,
            "ones_row": ones_row,
        })
    return in_maps


def _rerank(support_features, query_features, idx, mx8, ix8):
    """Exact re-rank of low-margin queries among their top-8 classes."""
    margin = mx8[:, 0] - mx8[:, 1]
    flagged = np.nonzero(margin < MARGIN_T)[0]
    if flagged.size == 0:
        return idx
    sf = np.asarray(support_features, np.float64).reshape(C, S, D)
    qf = np.asarray(query_features, np.float64)
    mean = sf.mean(axis=(0, 1), keepdims=True)
    s = sf - mean
    s /= np.linalg.norm(s, axis=2, keepdims=True)
    out = idx.copy()
    for q in flagged:
        qc = qf[q] - mean[0, 0]
        qc /= np.linalg.norm(qc)
        if mx8[q, 0] - mx8[q, 7] < DEEP_T:
            cands = np.arange(C)
        else:
            cands = ix8[q]
        best = (s[cands] @ qc).max(axis=1)
        out[q] = cands[int(np.argmax(best))]
    return out


def run(support_features, query_features, trace=False, **trace_kwargs):
    nc = build()
    in_maps = _host_inputs(support_features, query_features)
    res = run_bass_kernel_spmd(nc, in_maps, list(range(NCORES)),
                               trace=trace, **trace_kwargs)
    idx = np.concatenate(
        [np.asarray(rr["out"]).reshape(QS) for rr in res.results])
    mx8 = np.concatenate(
        [np.asarray(rr["out_mx8"]).reshape(QS, 8) for rr in res.results])
    ix8 = np.concatenate(
        [np.asarray(rr["out_ix8"]).reshape(QS, 8) for rr in res.results])
    out = _rerank(support_features, query_features, idx, mx8, ix8)
    return out.astype(np.int32), res


def kernel(support_features, query_features, use_cosine=None, **_ignored):
    out, _ = run(support_features, query_features, trace=False)
    return out
